# revision 23
# baseline (speedup 1.0000x reference)
"""Trainium2 Bass kernel for nn_EquivariantNetwork (EGNN message passing).

Strategy: data-parallel over batch (B=8 -> 8 NeuronCores, one sample per core).
Per core the N^2 edge stream (192x192 edges) is processed feature-on-partition
with two i-halves stacked on the 128 partitions:
  rows [edgA(0:32), corA(32:64), edgB(64:96), corB(96:128)]
Edge MLP matmuls run in float32r (full-rate PE). The h_i/h_j rank-structure of
the first edge layer is exploited: h_j / h_i contributions stream via
stride-0-broadcast access patterns (no N^2 materialization of h).
Pairwise d2/d0 are built as [96,192] Gram matrices via a K=5 matmul and
flatten-DMA'd into the edge stream. The e-gate diagonal is masked by a -60
logit contribution via an identity-pattern K=2 matmul. The coordinate update
runs in matrix layout ([96,192] tiles) where elementwise work is ~100x cheaper.
"""

import numpy as np

import concourse.bass as bass
import concourse.bacc as bacc
import concourse.mybir as mybir
from concourse.tile import TileContext, add_dep_helper
from concourse.bass_utils import run_bass_kernel_spmd

F32 = mybir.dt.float32
F32R = mybir.dt.float32r
AF = mybir.ActivationFunctionType
OP = mybir.AluOpType

B, N, HALF, F, L = 8, 192, 96, 32, 4
SEG = 2                # i-rows per chunk (per half)
CH = SEG * N           # 384 edge columns per chunk
NCH = HALF // SEG      # chunks per layer
SUB = CH               # columns per matmul (single sub-chunk)
PSW = CH               # psum tile width
T_DIFF = 1000.0
LN_EPS = 1e-5


def _softplus(x):
    return np.log1p(np.exp(-np.abs(x))) + np.maximum(x, 0.0)


def _sigmoid(x):
    return 1.0 / (1.0 + np.exp(-x))


# ---------------------------------------------------------------- host prep
def _host_prep(inp):
    f = np.float32
    sh = {}

    def hj(w):  # [32,66] -> h_j block transposed
        return w[:, F:2 * F].T.copy()

    def hi(w):
        return w[:, 0:F].T.copy()

    wj = np.zeros((L, F, 128), f)
    wi = np.zeros((L, 2 * F, 128), f)
    wd = np.zeros((L, 4, 128), f)
    w2 = np.zeros((L, 128, 128), f)
    w3 = np.zeros((L, 128, 66), f)
    wn1 = np.zeros((L, 128, 64), f)
    wn2 = np.zeros((L, 64, 64), f)
    b1 = np.zeros((128, L), f)
    b2 = np.zeros((128, L), f)
    bsig = np.zeros((64, L), f)
    bn1 = np.zeros((64, L), f)
    bn2 = np.zeros((64, L), f)
    c3b = np.zeros((2, L), f)
    for l in range(L):
        e1, c1 = inp['edg1_w'][l], inp['cor1_w'][l]
        wj[l] = np.hstack([hj(e1), hj(c1), hj(e1), hj(c1)])
        wi[l, 0:32, 0:64] = np.hstack([hi(e1), hi(c1)])
        wi[l, 32:64, 64:128] = np.hstack([hi(e1), hi(c1)])
        # d2d0 rows: [d2A, d0A, d2B, d0B]; cols [edgA, corA, edgB, corB]
        for blk, w in ((0, e1), (1, c1), (2, e1), (3, c1)):
            r0 = 0 if blk < 2 else 2
            wd[l, r0 + 0, blk * 32:(blk + 1) * 32] = w[:, 64]
            wd[l, r0 + 1, blk * 32:(blk + 1) * 32] = w[:, 65]
        e2, c2 = inp['edg2_w'][l], inp['cor2_w'][l]
        w2[l, 0:32, 0:32] = e2.T       # m2A <- m1edgA(rows0:32)
        w2[l, 64:96, 32:64] = e2.T     # m2B <- m1edgB(rows64:96)
        w2[l, 32:64, 64:96] = c2.T     # cw2A <- m1corA(rows32:64)
        w2[l, 96:128, 96:128] = c2.T   # cw2B
        ei, c3 = inp['edgi_w'][l][0], inp['cor3_w'][l][0]
        w3[l, 0:32, 0:32] = np.tile(ei[:, None], (1, 32))
        w3[l, 32:64, 32:64] = np.tile(ei[:, None], (1, 32))
        w3[l, 64:96, 64] = c3
        w3[l, 96:128, 65] = c3
        n1, n2w = inp['node1_w'][l], inp['node2_w'][l]
        wn1[l, 0:32, 0:32] = n1[:, 0:32].T
        wn1[l, 32:64, 0:32] = n1[:, 32:64].T
        wn1[l, 64:96, 32:64] = n1[:, 0:32].T
        wn1[l, 96:128, 32:64] = n1[:, 32:64].T
        wn2[l, 0:32, 0:32] = n2w.T
        wn2[l, 32:64, 32:64] = n2w.T
        b1[:, l] = np.concatenate([inp['edg1_b'][l], inp['cor1_b'][l],
                                   inp['edg1_b'][l], inp['cor1_b'][l]])
        b2[:, l] = np.concatenate([inp['edg2_b'][l], inp['edg2_b'][l],
                                   inp['cor2_b'][l], inp['cor2_b'][l]])
        bsig[:, l] = inp['edgi_b'][l][0] / 2.0
        bn1[:, l] = np.concatenate([inp['node1_b'][l], inp['node1_b'][l]])
        bn2[:, l] = np.concatenate([inp['node2_b'][l], inp['node2_b'][l]])
        c3b[:, l] = inp['cor3_b'][l][0]

    sh['wj'] = np.concatenate([wj[l] for l in range(L)], axis=1)      # [32,512]
    sh['wi'] = np.concatenate([wi[l] for l in range(L)], axis=1)      # [64,512]
    sh['wd'] = np.concatenate([wd[l] for l in range(L)], axis=1)      # [4,512]
    sh['w2'] = np.concatenate([w2[l] for l in range(L)], axis=1)      # [128,512]
    sh['w3'] = np.concatenate([w3[l] for l in range(L)], axis=1)      # [128,264]
    sh['wn1'] = np.concatenate([wn1[l] for l in range(L)], axis=1)    # [128,256]
    sh['wn2'] = np.concatenate([wn2[l] for l in range(L)], axis=1)    # [64,256]
    wm = np.zeros((34, 66), f)
    wm[32, 0:32] = -1e38
    wm[33, 32:64] = -1e38
    sh['wm'] = wm
    sh['eyeA'] = np.eye(96, 192, 0, dtype=f)
    sh['eyeB'] = np.eye(96, 192, 96, dtype=f)
    sh['b1'], sh['b2'], sh['bsig'] = b1, b2, bsig
    sh['bn1'], sh['bn2'], sh['c3b'] = bn1, bn2, c3b
    sh['lnhg'] = inp['lnh_g'].T.astype(f).copy()   # [32,4]
    sh['lnhb'] = inp['lnh_b'].T.astype(f).copy()
    sh['lnxg'] = inp['lnx_g'].T.astype(f).copy()   # [3,4]
    sh['lnxb'] = inp['lnx_b'].T.astype(f).copy()

    # gamma MLP folds (all weight-derived)
    sp1w = _softplus(inp['g_l1_w'][0, 0].astype(f))
    b1g = inp['g_l1_b'][0].astype(f)
    spl2 = _softplus(inp['g_l2_w'][:, 0].astype(f))
    b2g = inp['g_l2_b'].astype(f)
    spl3 = _softplus(inp['g_l3_w'][0, :].astype(f))

    def gtilde(u):
        l1 = sp1w * u + b1g
        return l1 + np.dot(spl3, _sigmoid(spl2 * l1 + b2g))

    gt0, gt1 = gtilde(f(0.0)), gtilde(f(1.0))
    g0, g1 = inp['gamma_0'][0].astype(f), inp['gamma_1'][0].astype(f)
    Bc = (g1 - g0) / (gt1 - gt0)
    Ac = g0 - Bc * gt0
    sh['gp'] = np.array([[sp1w, b1g, Bc, Ac]], f)                    # [1,4]
    sh['spl2t'] = spl2.reshape(8, 128).T.copy()                      # [128,8]
    sh['spl3t'] = spl3.reshape(8, 128).T.copy()
    sh['b2t'] = b2g.reshape(8, 128).T.copy()

    pwp = np.zeros((10, 3072), f)
    pwp[:, :3000] = inp['pdf_w']
    pdfw = np.zeros((128, 240), f)
    for k in range(24):
        pdfw[:, k * 10:(k + 1) * 10] = pwp[:, k * 128:(k + 1) * 128].T
    sh['pdfw'] = pdfw
    emb = inp['emb_in_w'].astype(f)                                  # [32,12]
    sh['wembH'] = emb[:, 0:1].T.copy()                               # [1,32]
    sh['wembT'] = emb[:, 1:2].T.copy()
    sh['wembP'] = emb[:, 2:12].T.copy()                              # [10,32]
    sh['embb'] = inp['emb_in_b'].astype(f).reshape(32, 1)
    sh['weo'] = inp['emb_out_w'].astype(f).T.copy()                  # [32,1]
    sh['beo'] = inp['emb_out_b'].astype(f).reshape(1, 1)
    sh['wxo'] = inp['x_out_w'].astype(f).T.copy()                    # [3,3]
    sh['bxo'] = inp['x_out_b'].astype(f).reshape(3, 1)

    percore = []
    for s in range(B):
        d = {}
        d['xyzt'] = np.ascontiguousarray(inp['xyz_atom_species'][s].T.astype(f))
        pp = np.zeros(3072, f)
        pp[:3000] = inp['pdf'][s]
        d['pdft'] = pp.reshape(24, 128).T.copy()
        d['tns'] = np.array([[inp['t'][s] / T_DIFF]], f)
        percore.append(d)
    return sh, percore


_SHARED_SPECS = [
    ('wj', (32, 512), F32R), ('wi', (64, 512), F32R), ('wd', (4, 512), F32R),
    ('w2', (128, 512), F32R), ('w3', (128, 264), F32R), ('wm', (34, 66), F32R),
    ('wn1', (128, 256), F32R), ('wn2', (64, 256), F32R),
    ('eyeA', (96, 192), F32R), ('eyeB', (96, 192), F32R),
    ('b1', (128, 4), F32), ('b2', (128, 4), F32), ('bsig', (64, 4), F32),
    ('bn1', (64, 4), F32), ('bn2', (64, 4), F32), ('c3b', (2, 4), F32),
    ('lnhg', (32, 4), F32), ('lnhb', (32, 4), F32),
    ('lnxg', (3, 4), F32), ('lnxb', (3, 4), F32),
    ('gp', (1, 4), F32), ('spl2t', (128, 8), F32), ('spl3t', (128, 8), F32),
    ('b2t', (128, 8), F32), ('pdfw', (128, 240), F32),
    ('wembH', (1, 32), F32), ('wembT', (1, 32), F32), ('wembP', (10, 32), F32),
    ('embb', (32, 1), F32), ('weo', (32, 1), F32), ('beo', (1, 1), F32),
    ('wxo', (3, 3), F32), ('bxo', (3, 1), F32),
]
_DATA_SPECS = [('xyzt', (4, 192), F32), ('pdft', (128, 24), F32),
               ('tns', (1, 1), F32)]


# ---------------------------------------------------------------- program
def build_program():
    nc = bacc.Bacc("TRN2", target_bir_lowering=False, debug=False,
                   enable_asserts=True)
    dram = {}
    for name, shape, dt in _SHARED_SPECS + _DATA_SPECS:
        dram[name] = nc.dram_tensor(name, list(shape), dt, kind="ExternalInput").ap()
    out_d = nc.dram_tensor("out", [4, 192], F32, kind="ExternalOutput").ap()

    with TileContext(nc) as tc:
        with (
            tc.tile_pool(name="const", bufs=1) as cp,
            tc.tile_pool(name="mat", bufs=2) as mp,
            tc.tile_pool(name="stream", bufs=3) as sp,
            tc.tile_pool(name="s2p", bufs=3) as s2p,
            tc.tile_pool(name="ps1", bufs=3, space="PSUM") as ps1,
            tc.tile_pool(name="ps2", bufs=2, space="PSUM") as ps2,
            tc.tile_pool(name="ps3", bufs=2, space="PSUM") as ps3,
        ):
            # ---- load constants
            W = {}
            for name, shape, dt in _SHARED_SPECS + _DATA_SPECS:
                if name in ("eyeA", "eyeB"):
                    continue
                W[name] = cp.tile(list(shape), dt, tag=name, name='W_' + name)
                nc.sync.dma_start(W[name][:, :], dram[name])

            # static full-layer edge tensor: rows 0-3 = [d2A,d0A,d2B,d0B],
            # rows 32-33 = [eyeA,eyeB] (partition-32-aligned for matmul rhs)
            E = HALF * N
            est = cp.tile([34, E], F32R, tag="est", name="est")
            NPC = 4  # fill-piece DMAs per row
            for pc in range(NPC):
                r0 = pc * (HALF // NPC)
                c0 = pc * (E // NPC)
                cw = E // NPC
                nc.sync.dma_start(est[32:33, c0:c0 + cw],
                                    dram['eyeA'][r0:r0 + HALF // NPC, :])
                nc.sync.dma_start(est[33:34, c0:c0 + cw],
                                    dram['eyeB'][r0:r0 + HALF // NPC, :])

            # static aux tiles (rewritten per layer; statics keep DMA waits <= 1)
            cwmA = cp.tile([96, 192], F32, tag="cwmA", name="cwmA")
            cwmB = cp.tile([96, 192], F32, tag="cwmB", name="cwmB")
            dxt = cp.tile([3, 192], F32, tag="dxt", name="dxt")
            augL = cp.tile([5, 192], F32, tag="augL", name="augL")
            augR = cp.tile([5, 192], F32, tag="augR", name="augR")
            agDs = [cp.tile([2, 192], F32, tag=f"agD{d}", name=f"agD{d}")
                    for d in range(3)]
            agRs = [cp.tile([2, 192], F32, tag=f"agR{d}", name=f"agR{d}")
                    for d in range(3)]

            inv32 = cp.tile([32, 1], F32, tag="inv32")
            nc.vector.memset(inv32[:, :], 1.0 / 32.0)
            inv3 = cp.tile([3, 1], F32, tag="inv3")
            nc.vector.memset(inv3[:, :], 1.0 / 3.0)
            rv32 = cp.tile([1, 32], F32, tag="rv32")
            nc.vector.memset(rv32[:, :], 1.0)
            rv3 = cp.tile([1, 3], F32, tag="rv3")
            nc.vector.memset(rv3[:, :], 1.0)
            rv128 = cp.tile([1, 128], F32, tag="rv128")
            nc.vector.memset(rv128[:, :], 1.0)
            o128 = cp.tile([128, 1], F32, tag="o128")
            nc.vector.memset(o128[:, :], 1.0)
            o3 = cp.tile([3, 1], F32, tag="o3")
            nc.vector.memset(o3[:, :], 1.0)
            epsT = cp.tile([1, 1], F32, tag="epsT")
            nc.vector.memset(epsT[:, :], LN_EPS)
            onesrow = cp.tile([1, 192], F32, tag="onesrow")
            nc.vector.memset(onesrow[:, :], 1.0)

            xt = W['xyzt']           # [4,192]: rows 0-2 = x0, row 3 = h_raw
            x0 = xt[0:3, :]

            # ---- t embedding (scalar chain)
            l1t = mp.tile([1, 1], F32, tag="sc1")
            nc.vector.tensor_scalar(l1t[:, :], W['tns'][:, :],
                                    W['gp'][:, 0:1], W['gp'][:, 1:2],
                                    OP.mult, OP.add)
            psl = ps1.tile([128, 192], F32, tag="big")
            nc.tensor.matmul(psl[:, 0:1], rv128[:, :], l1t[:, :],
                             start=True, stop=True)
            l1c = mp.tile([128, 1], F32, tag="l1c")
            nc.vector.tensor_copy(l1c[:, :], psl[:, 0:1])
            qt = mp.tile([128, 8], F32, tag="qt")
            nc.vector.tensor_scalar(qt[:, :], W['spl2t'][:, :], l1c[:, :], None,
                                    OP.mult)
            nc.vector.tensor_add(qt[:, :], qt[:, :], W['b2t'][:, :])
            nc.scalar.activation(qt[:, :], qt[:, :], AF.Sigmoid)
            nc.vector.tensor_mul(qt[:, :], qt[:, :], W['spl3t'][:, :])
            rs = mp.tile([128, 1], F32, tag="rs")
            nc.vector.tensor_reduce(rs[:, :], qt[:, :], axis=mybir.AxisListType.X,
                                    op=OP.add)
            pst = ps2.tile([128, PSW], F32, tag="big2")
            nc.tensor.matmul(pst[0:1, 0:1], o128[:, :], rs[:, :],
                             start=True, stop=True)
            gts = mp.tile([1, 1], F32, tag="gts")
            nc.vector.tensor_add(gts[:, :], l1t[:, :], pst[0:1, 0:1])
            temb = mp.tile([1, 1], F32, tag="temb")
            nc.vector.tensor_scalar(temb[:, :], gts[:, :],
                                    W['gp'][:, 2:3], W['gp'][:, 3:4],
                                    OP.mult, OP.add)

            # ---- pdf embedding
            psp_ = ps3.tile([66, PSW], F32, tag="big3")
            for k in range(24):
                nc.tensor.matmul(psp_[0:10, 0:1],
                                 W['pdfw'][:, k * 10:(k + 1) * 10],
                                 W['pdft'][:, k:k + 1],
                                 start=(k == 0), stop=(k == 23))
            pe = mp.tile([10, 1], F32, tag="pe")
            nc.vector.tensor_copy(pe[:, :], psp_[0:10, 0:1])

            # ---- h0
            psb = ps1.tile([128, 192], F32, tag="big")
            nc.tensor.matmul(psb[0:32, 0:1], W['wembT'][:, :], temb[:, :],
                             start=True, stop=False)
            nc.tensor.matmul(psb[0:32, 0:1], W['wembP'][:, :], pe[:, :],
                             start=False, stop=True)
            hbb = mp.tile([32, 1], F32, tag="hbb")
            nc.vector.tensor_add(hbb[:, :], psb[0:32, 0:1], W['embb'][:, :])
            hraw = mp.tile([1, 192], F32, tag="hraw")
            nc.sync.dma_start(hraw[:, :], dram['xyzt'][3:4, :])
            psh = ps2.tile([128, PSW], F32, tag="big2")
            nc.tensor.matmul(psh[0:32, 0:192], W['wembH'][:, :], hraw[:, :],
                             start=True, stop=True)
            h_cur = mp.tile([32, 192], F32, tag="h")
            nc.scalar.activation(h_cur[:, :], psh[0:32, 0:192], AF.Identity,
                                 bias=hbb[:, :], scale=1.0)

            # ---- d0 matrices (from x0)
            def gram(x_ap, d2A_t, d2B_t, sqrt_to=None, recip_to=None):
                """d2 Gram matrices per half; optional sqrt(max(d2,1e-12))
                into sqrt_to[hh] and 1/(sqrt+1) into recip_to[hh]."""
                xsq = mp.tile([3, 192], F32, tag="xsq")
                nc.scalar.activation(xsq[:, :], x_ap, AF.Square)
                psn = ps1.tile([128, 192], F32, tag="big")
                nc.tensor.matmul(psn[0:1, 0:192], o3[:, :], xsq[:, :],
                                 start=True, stop=True)
                n2 = mp.tile([1, 192], F32, tag="n2")
                nc.vector.tensor_copy(n2[:, :], psn[0:1, 0:192])
                augL = mp.tile([5, 192], F32, tag="augL")
                nc.vector.tensor_scalar_mul(augL[0:3, :], x_ap, -2.0)
                nc.sync.dma_start(augL[3:4, :], n2[:, :])
                nc.sync.dma_start(augL[4:5, :], onesrow[:, :])
                augR = mp.tile([5, 192], F32, tag="augR")
                nc.vector.tensor_copy(augR[0:3, :], x_ap)
                nc.sync.dma_start(augR[3:4, :], onesrow[:, :])
                nc.sync.dma_start(augR[4:5, :], n2[:, :])
                for hh, d2t in ((0, d2A_t), (1, d2B_t)):
                    psd = ps1.tile([128, 192], F32, tag="big")
                    nc.tensor.matmul(psd[0:96, 0:192],
                                     augL[:, hh * 96:(hh + 1) * 96],
                                     augR[:, :], start=True, stop=True)
                    if d2t is not None:
                        nc.vector.tensor_copy(d2t[:, :], psd[0:96, 0:192])
                    if sqrt_to is not None:
                        dcl = mp.tile([96, 192], F32, tag="dcl")
                        nc.vector.tensor_scalar_max(dcl[:, :], psd[0:96, 0:192],
                                                    1e-12)
                        nc.scalar.activation(sqrt_to[hh][:, :], dcl[:, :],
                                             AF.Sqrt)
                    if recip_to is not None:
                        dp1 = mp.tile([96, 192], F32, tag="dp1")
                        nc.vector.tensor_scalar_add(
                            dp1[:, :], sqrt_to[hh][:, :].bitcast(F32), 1.0)
                        nc.vector.reciprocal(recip_to[hh][:, :], dp1[:, :])

            d0mA = cp.tile([96, 192], F32R, tag="d0mA")
            d0mB = cp.tile([96, 192], F32R, tag="d0mB")
            gram(x0, None, None, sqrt_to=[d0mA, d0mB])
            for pc in range(NPC):
                r0 = pc * (HALF // NPC)
                c0 = pc * (E // NPC)
                cw = E // NPC
                nc.sync.dma_start(est[1:2, c0:c0 + cw],
                                    d0mA[r0:r0 + HALF // NPC, :])
                nc.sync.dma_start(est[3:4, c0:c0 + cw],
                                    d0mB[r0:r0 + HALF // NPC, :])

            x_cur = x0  # layer-0 x
            x_pending = None  # (x_ln, dxt) awaiting post-barrier add

            # ================================================= layer loop
            for l in range(L):
                tc.strict_bb_all_engine_barrier()
                if x_pending is not None:
                    x_new = mp.tile([3, 192], F32, tag="x")
                    nc.vector.tensor_add(x_new[:, :], x_pending[0][:, :],
                                         x_pending[1][:, :])
                    x_cur = x_new
                # d2 matrices + dist recip (pre-LN x)
                d2mA = mp.tile([96, 192], F32R, tag="d2mA")
                d2mB = mp.tile([96, 192], F32R, tag="d2mB")
                dsA = mp.tile([96, 192], F32, tag="dsA")
                dsB = mp.tile([96, 192], F32, tag="dsB")
                rA = mp.tile([96, 192], F32, tag="rA")
                rB = mp.tile([96, 192], F32, tag="rB")
                gram(x_cur[:, :], d2mA, d2mB, sqrt_to=[dsA, dsB],
                     recip_to=[rA, rB])
                for pc in range(NPC):
                    r0 = pc * (HALF // NPC)
                    c0 = pc * (E // NPC)
                    cw = E // NPC
                    nc.sync.dma_start(est[0:1, c0:c0 + cw],
                                        d2mA[r0:r0 + HALF // NPC, :])
                    nc.sync.dma_start(est[2:3, c0:c0 + cw],
                                        d2mB[r0:r0 + HALF // NPC, :])

                # diff matrices * r -> diffn [96,192] x 6
                negx = mp.tile([3, 192], F32, tag="negx")
                nc.vector.tensor_scalar_mul(negx[:, :], x_cur[:, :], -1.0)
                diffn = []
                for d in range(3):
                    agD = agDs[d]
                    nc.vector.memset(agD[:, :], 1.0)
                    nc.sync.dma_start(agD[0:1, :], x_cur[d:d + 1, :])
                    agR = agRs[d]
                    nc.vector.memset(agR[:, :], 1.0)
                    nc.sync.dma_start(agR[1:2, :], negx[d:d + 1, :])
                    row = []
                    for hh, rr in ((0, rA), (1, rB)):
                        psf = ps1.tile([128, 192], F32, tag="big")
                        nc.tensor.matmul(psf[0:96, 0:192],
                                         agD[:, hh * 96:(hh + 1) * 96],
                                         agR[:, :], start=True, stop=True)
                        dn = mp.tile([96, 192], F32, tag=f"dn{d}{hh}")
                        nc.vector.tensor_mul(dn[:, :], rr[:, :], psf[0:96, 0:192])
                        row.append(dn)
                    diffn.append(row)

                # ---- LayerNorm h -> h_ln, hr(f32r); LayerNorm x -> x_ln
                def layer_norm(src, P_, invP, rvP, g_ap, b_ap, tagp):
                    psm = ps1.tile([128, 192], F32, tag="big")
                    nc.tensor.matmul(psm[0:1, 0:192], invP[:, :], src,
                                     start=True, stop=True)
                    mean = mp.tile([1, 192], F32, tag=f"mean{tagp}")
                    nc.vector.tensor_copy(mean[:, :], psm[0:1, 0:192])
                    psr = ps1.tile([128, 192], F32, tag="big")
                    nc.tensor.matmul(psr[0:P_, 0:192], rvP[:, :], mean[:, :],
                                     start=True, stop=True)
                    hc = mp.tile([P_, 192], F32, tag=f"hc{tagp}")
                    nc.vector.tensor_sub(hc[:, :], src, psr[0:P_, 0:192])
                    hc2 = mp.tile([P_, 192], F32, tag=f"hc2{tagp}")
                    nc.scalar.activation(hc2[:, :], hc[:, :], AF.Square)
                    psv = ps1.tile([128, 192], F32, tag="big")
                    nc.tensor.matmul(psv[0:1, 0:192], invP[:, :], hc2[:, :],
                                     start=True, stop=True)
                    sd = mp.tile([1, 192], F32, tag=f"sd{tagp}")
                    nc.scalar.activation(sd[:, :], psv[0:1, 0:192], AF.Sqrt,
                                         bias=epsT[:, :], scale=1.0)
                    rstd = mp.tile([1, 192], F32, tag=f"rstd{tagp}")
                    nc.vector.reciprocal(rstd[:, :], sd[:, :])
                    pss = ps1.tile([128, 192], F32, tag="big")
                    nc.tensor.matmul(pss[0:P_, 0:192], rvP[:, :], rstd[:, :],
                                     start=True, stop=True)
                    xn = mp.tile([P_, 192], F32, tag=f"xn{tagp}")
                    nc.vector.tensor_mul(xn[:, :], hc[:, :], pss[0:P_, 0:192])
                    outl = mp.tile([P_, 192], F32, tag=f"ln{tagp}")
                    nc.scalar.activation(outl[:, :], xn[:, :], AF.Identity,
                                         bias=b_ap, scale=g_ap)
                    return outl

                h_ln = layer_norm(h_cur[:, :], 32, inv32, rv32,
                                  W['lnhg'][:, l:l + 1], W['lnhb'][:, l:l + 1],
                                  "h")
                hr = mp.tile([64, 192], F32R, tag="hr")
                nc.vector.tensor_copy(hr[0:32, :], h_ln[:, :])
                nc.vector.tensor_copy(hr[32:64, 0:96], h_ln[:, 96:192])
                x_ln = layer_norm(x_cur[:, :], 3, inv3, rv3,
                                  W['lnxg'][:, l:l + 1], W['lnxb'][:, l:l + 1],
                                  "x")

                agg = mp.tile([64, 96], F32, tag="agg")
                tc.strict_bb_all_engine_barrier()

                # ---------------------------- edge-stream chunk loop
                for c in range(NCH):
                    c4 = c * SEG
                    p1 = ps1.tile([128, PSW], F32, tag="big")
                    nc.tensor.matmul(
                        p1[:, :].rearrange("p (a b) -> p a b", a=SEG),
                        W['wj'][:, 128 * l:128 * l + 128],
                        hr[0:32, None, :].broadcast_to([32, SEG, 192]),
                        start=True, stop=False)
                    nc.tensor.matmul(
                        p1[:, :].rearrange("p (a b) -> p a b", a=SEG),
                        W['wi'][:, 128 * l:128 * l + 128],
                        hr[:, c4:c4 + SEG][:, :, None].broadcast_to([64, SEG, 192]),
                        start=False, stop=False)
                    nc.tensor.matmul(
                        p1[:, :],
                        W['wd'][:, 128 * l:128 * l + 128],
                        est[0:4, c * CH:(c + 1) * CH],
                        start=False, stop=True)
                    s1 = sp.tile([128, CH], F32R, tag="s1")
                    nc.scalar.activation(s1[:, :], p1[:, :],
                                         AF.Silu, bias=W['b1'][:, l:l + 1],
                                         scale=1.0)

                    p2 = ps2.tile([128, PSW], F32, tag="big2")
                    nc.tensor.matmul(p2[:, :],
                                     W['w2'][:, 128 * l:128 * l + 128],
                                     s1[:, :], start=True, stop=True)
                    s2 = s2p.tile([128, CH], F32R, tag="s2")
                    nc.scalar.activation(s2[:, :], p2[:, :],
                                         AF.Silu, bias=W['b2'][:, l:l + 1],
                                         scale=1.0)

                    p3 = ps3.tile([66, PSW], F32, tag="big3")
                    nc.tensor.matmul(p3[:, :],
                                     W['w3'][:, 66 * l:66 * l + 66],
                                     s2[:, :], start=True, stop=False)
                    nc.tensor.matmul(
                        p3[:, :], W['wm'][32:34, :],
                        est[32:34, c * CH:(c + 1) * CH],
                        start=False, stop=True)
                    et = sp.tile([64, CH], F32, tag="et")
                    nc.scalar.activation(et[:, :], p3[0:64, :],
                                         AF.Tanh, bias=W['bsig'][:, l:l + 1],
                                         scale=0.5)
                    et2 = sp.tile([64, CH], F32, tag="et2")
                    nc.gpsimd.tensor_scalar(et2[:, :], et[:, :], 0.5, 0.5,
                                            OP.mult, OP.add)
                    em = sp.tile([64, CH], F32, tag="em")
                    nc.vector.tensor_mul(em[:, :], et2[:, :],
                                         s2[0:64, :].bitcast(F32))
                    cwe = sp.tile([2, CH], F32, tag="cwe")
                    nc.vector.tensor_scalar_add(cwe[:, :], p3[64:66, :],
                                                W['c3b'][:, l:l + 1])
                    nc.vector.tensor_reduce(
                        agg[:, c4:c4 + SEG],
                        em[:, :].rearrange("p (a b) -> p a b", a=SEG),
                        axis=mybir.AxisListType.X, op=OP.add)
                    nc.sync.dma_start(cwmA[c4:c4 + SEG, :], cwe[0:1, :])
                    nc.sync.dma_start(cwmB[c4:c4 + SEG, :], cwe[1:2, :])

                # ---------------------------- coordinate update
                for hh, cwm in ((0, cwmA), (1, cwmB)):
                    for d in range(3):
                        wdm = mp.tile([96, 192], F32, tag="wdm")
                        nc.vector.tensor_mul(wdm[:, :], cwm[:, :],
                                             diffn[d][hh][:, :])
                        dx1 = mp.tile([96, 1], F32, tag="dx1")
                        nc.vector.tensor_reduce(dx1[:, :], wdm[:, :],
                                                axis=mybir.AxisListType.X,
                                                op=OP.add)
                        nc.sync.dma_start(dxt[d:d + 1, hh * 96:(hh + 1) * 96],
                                          dx1[:, :])
                x_pending = (x_ln, dxt)

                # ---------------------------- node MLP
                cat = mp.tile([128, 96], F32R, tag="cat")
                nc.vector.tensor_copy(cat[0:32, :], hr[0:32, 0:96].bitcast(F32))
                nc.vector.tensor_copy(cat[32:64, :], agg[0:32, :])
                nc.vector.tensor_copy(cat[64:96, :], hr[0:32, 96:192].bitcast(F32))
                nc.vector.tensor_copy(cat[96:128, :], agg[32:64, :])
                psn1 = ps1.tile([128, 192], F32, tag="big")
                nc.tensor.matmul(psn1[0:64, 0:96],
                                 W['wn1'][:, 64 * l:64 * l + 64], cat[:, :],
                                 start=True, stop=True)
                hn = mp.tile([64, 96], F32R, tag="hn")
                nc.scalar.activation(hn[:, :], psn1[0:64, 0:96], AF.Silu,
                                     bias=W['bn1'][:, l:l + 1], scale=1.0)
                psn2 = ps1.tile([128, 192], F32, tag="big")
                nc.tensor.matmul(psn2[0:64, 0:96],
                                 W['wn2'][:, 64 * l:64 * l + 64],
                                 hn[:, :], start=True, stop=True)
                hlb = mp.tile([32, 192], F32, tag="hlb")
                nc.vector.tensor_scalar_add(hlb[:, :], h_ln[:, :],
                                            W['bn2'][0:32, l:l + 1])
                h_new = mp.tile([32, 192], F32, tag="h")
                nc.vector.tensor_add(h_new[:, 0:96], hlb[:, 0:96],
                                     psn2[0:32, 0:96])
                nc.vector.tensor_add(h_new[:, 96:192], hlb[:, 96:192],
                                     psn2[32:64, 0:96])

                h_cur = h_new

            # ================================================= epilogue
            x_fin = mp.tile([3, 192], F32, tag="x")
            nc.vector.tensor_add(x_fin[:, :], x_pending[0][:, :],
                                 x_pending[1][:, :])
            x_cur = x_fin
            pse1 = ps1.tile([128, 192], F32, tag="big")
            nc.tensor.matmul(pse1[0:1, 0:192], W['weo'][:, :], h_cur[:, :],
                             start=True, stop=True)
            pse2 = ps1.tile([128, 192], F32, tag="big")
            nc.tensor.matmul(pse2[0:3, 0:192], W['wxo'][:, :], x_cur[:, :],
                             start=True, stop=True)
            oxr = mp.tile([3, 192], F32, tag="oxr")
            nc.scalar.activation(oxr[:, :], pse2[0:3, 0:192], AF.Identity,
                                 bias=W['bxo'][:, :], scale=1.0)
            ox2 = mp.tile([3, 192], F32, tag="ox2")
            nc.vector.tensor_sub(ox2[:, :], oxr[:, :], x0)
            hout = mp.tile([1, 192], F32, tag="hout")
            nc.scalar.activation(hout[:, :], pse1[0:1, 0:192], AF.Identity,
                                 bias=W['beo'][:, :], scale=1.0)
            nc.sync.dma_start(out_d[0:3, :], ox2[:, :])
            nc.sync.dma_start(out_d[3:4, :], hout[:, :])
    nc.compile()
    return nc


_PROGRAM = None


def _get_program():
    global _PROGRAM
    if _PROGRAM is None:
        _PROGRAM = build_program()
    return _PROGRAM


def _run(inputs, trace=False):
    inputs = {k: np.asarray(v) for k, v in inputs.items()}
    sh, percore = _host_prep(inputs)
    nc = _get_program()
    in_maps = []
    for s in range(B):
        m = dict(sh)
        m.update(percore[s])
        in_maps.append(m)
    res = run_bass_kernel_spmd(nc, in_maps, core_ids=list(range(B)), trace=trace)
    outs = []
    for s in range(B):
        o = res.results[s]["out"]          # [4,192]
        outs.append(o.T)                   # -> [192,4] cols [x(3), h(1)]
    return np.stack(outs, axis=0).astype(np.float32), res


def kernel(**inputs):
    out, _ = _run(inputs)
    return out


if __name__ == "__main__":
    # smoke build
    nc = build_program()
    print("built OK")
